# revision 5
# baseline (speedup 1.0000x reference)
"""Single-head causal attention (B=128, T=512, C=256, H=64) on 8 trn2 cores.

Data-parallel: 16 batches per core. Host pre-transposes/casts x to bf16
xT [C, T] per batch (host prep is free), so the device does no transposes
and no cast-DMAs. Per batch b, software-pipelined across slots:

  slot b   : load xT (HWDGE, ramped 1/1/2/2.. batch chunks);
             pqk = [Wq|Wk]^T @ xT in two 256-col halves (psum slack regions
             of both PSUM halves); pv = xT-chunks @ Wv (psum bank3/bank7
             slack); qk copy -> qk2 pair tile (DVE); v1 copy (DVE) into
             [128,4,65] tile whose 65th column is set to 1.0 (Pool memset)
             so the AV matmuls produce the softmax denominator for free.
  odd slots: one SBUF->SBUF DMA shifts the k-halves of BOTH batches of the
             qk2 pair to partitions 0-63 (PE needs stationary+moving at the
             same base partition).
  slot b+4 : 7 sim matmuls -> packed block-causal psim [128,1280] fp32
             (diags | si0-od | si2-od | si1-od, every region inside one
             2KB bank); one ACT exp over all 1280 cols -> pT bf16; lower
             triangle of the 4 diag blocks zeroed in-place via gpsimd
             affine_select (iota = t - s).
  slot b+6 : 10 AV matmuls (pT blocks stationary, v1 [128,65] moving) ->
             pav [128,4,65] incl. rowsum column.
  slot b+8 : (even pairs) one DVE drain of both parities' pav -> osb
             [128,2,4,65] bf16; fully-contiguous out DMA (1040B/partition
             descriptors). Host unscrambles, divides by the rowsum column
             and upcasts to fp32.

PSUM is one [128, 2, 2048] tile sliced manually (subtile byte-range
fencing): per half h: psim at cols 0:1280 (banks 4h..4h+2 lower), pqk
half at 1280:1536 (bank 4h+2 upper), pv scpair at 1536:1664 (bank 4h+3),
pav at 1664:1924 (bank 4h+3). Every matmul output region sits inside a
single 2KB bank.  An ACT exp warmup at t=0 pulls the 1.3us
activation-table load into the initial DMA dead time.
"""
import numpy as np
import ml_dtypes

B, T, C, H = 128, 512, 256, 64
N_CORES = 8
BL = B // N_CORES          # batches per core
TC = T // 128              # 4 t-chunks
CS = C // 128              # 2 c-subtiles
INV_SQRT_H = 1.0 / np.sqrt(H)

XCH = 2                    # batches per x-load DMA
SIM_LAG = 4                # sim of batch b runs in slot b+SIM_LAG
AV_LAG = 6                 # AV of batch b runs in slot b+AV_LAG
DRAIN_LAG = 8              # drain of pair (b, b+1) runs in slot b+DRAIN_LAG

# psim packed column layout inside a half: diags@0 od0@512 od2@896 od1@1024
OD0, OD2, OD1 = 512, 896, 1024
PQK_OFF = 1280             # [1280:1536] pqk T-half
PV_OFF = 1536              # [1536:1664] pv (2 of 4 s-chunks per half)
PAV_OFF = 1664             # [1664:1924] pav [4, 65]


def _blk(si, ci):
    """psim/pT column offset of [128,128] block (s-chunk si, t-chunk ci)."""
    if si == ci:
        return 128 * si
    if si == 0:
        return OD0 + 128 * (ci - 1)
    if si == 1:
        return OD1 + 128 * (ci - 2)
    assert si == 2 and ci == 3
    return OD2


def _build_program():
    import concourse.tile as tile
    from concourse import bacc, mybir

    dt = mybir.dt
    nc = bacc.Bacc("TRN2", target_bir_lowering=False, debug=False,
                   enable_asserts=False, num_devices=N_CORES)

    xt_d = nc.dram_tensor("xt", [BL, CS, 128, T], dt.bfloat16,
                          kind="ExternalInput").ap()
    wqk_d = nc.dram_tensor("wqk", [CS, 128, 128], dt.bfloat16,
                           kind="ExternalInput").ap()
    wv_d = nc.dram_tensor("wv", [CS, 128, H], dt.bfloat16,
                          kind="ExternalInput").ap()
    out_d = nc.dram_tensor("out", [BL // 2, 128, 2, TC, H + 1], dt.bfloat16,
                           kind="ExternalOutput").ap()

    with tile.TileContext(nc) as tc:
        from contextlib import ExitStack
        ctx = ExitStack()
        with ctx:
            consts = ctx.enter_context(tc.tile_pool(name="consts", bufs=1))
            ps = ctx.enter_context(tc.tile_pool(name="ps", bufs=1,
                                                space="PSUM"))
            # whole PSUM as one tile, sliced manually (byte-range fencing)
            pt_all = ps.tile([128, 2, 2048], dt.float32, name="psall")
            sb_x = ctx.enter_context(tc.tile_pool(name="sb_x", bufs=3))
            sb_qk = ctx.enter_context(tc.tile_pool(name="sb_qk", bufs=4))
            sb_p = ctx.enter_context(tc.tile_pool(name="sb_p", bufs=4))
            sb_v = ctx.enter_context(tc.tile_pool(name="sb_v", bufs=8))
            sb_o = ctx.enter_context(tc.tile_pool(name="sb_o", bufs=3))

            # ACT activation-table warmup so the implicit table load happens
            # during the initial DMA dead-time, not before the first real exp.
            warm = consts.tile([128, 1], dt.float32)
            nc.gpsimd.memset(warm[:], 0.0)
            warm2 = consts.tile([128, 1], dt.bfloat16)
            nc.scalar.activation(warm2[:], warm[:],
                                 mybir.ActivationFunctionType.Exp, scale=1.0)

            wqk_sb = consts.tile([128, CS, 128], dt.bfloat16)
            nc.sync.dma_start(wqk_sb[:], wqk_d.rearrange("cs p m -> p cs m"))
            wv_sb = consts.tile([128, CS, H], dt.bfloat16)
            nc.sync.dma_start(wv_sb[:], wv_d.rearrange("cs p h -> p cs h"))

            # manual psum slices
            pqk2 = pt_all[:, :, PQK_OFF:PQK_OFF + 256]        # [128, 2, 256]
            pv2 = pt_all[:, :, PV_OFF:PV_OFF + 128]           # [128, 2, 128]
            pv4 = pv2.rearrange("p a (s c) -> p a s c", s=2)  # [128,2,2,64]
            pavd = pt_all[:, :, PAV_OFF:PAV_OFF + 260].rearrange(
                "p a (tc h) -> p a tc h", tc=TC)              # [128,2,4,65]

            xts = [None] * BL
            qk2s = [None] * (BL // 2)
            kt2s = [None] * (BL // 2)
            pts = [None] * BL
            v1s = [None] * BL

            # x-load chunks: [1,1] first (fast pipeline fill), then pairs
            chunks = [(0, 1), (1, 2)] + [(c, c + 2) for c in range(2, BL, 2)]
            chunk_issue_slot = [max(0, lo - 2) for lo, hi in chunks]

            n_slots = BL + DRAIN_LAG - 1
            next_chunk = 0
            for k in range(n_slots):
                # ---- x prefetch
                while (next_chunk < len(chunks)
                       and chunk_issue_slot[next_chunk] <= k):
                    lo, hi = chunks[next_chunk]
                    xt2 = sb_x.tile([128, XCH, CS, T], dt.bfloat16,
                                    name=f"xt2_{next_chunk}", tag="xt2")
                    nc.sync.dma_start(
                        xt2[:, 0:hi - lo],
                        xt_d[lo:hi].rearrange("b cs p t -> p b cs t"))
                    for j in range(hi - lo):
                        xts[lo + j] = xt2[:, j]
                    next_chunk += 1

                # ---- drain pair (b, b+1), b = k - DRAIN_LAG (even slots)
                if k >= DRAIN_LAG and (k - DRAIN_LAG) % 2 == 0:
                    q2 = (k - DRAIN_LAG) // 2
                    osb = sb_o.tile([128, 2, TC, H + 1], dt.bfloat16,
                                    name=f"osb{q2}", tag="osb")
                    nc.vector.tensor_copy(osb[:], pavd)
                    nc.sync.dma_start(out_d[q2], osb[:])

                # ---- stage P1 (batch k): qk + v matmuls, copies, ones
                if k < BL:
                    b = k
                    r = b % 2
                    xt = xts[b]
                    for h2 in range(2):
                        for cc in range(CS):
                            nc.tensor.matmul(
                                pqk2[:, h2, :], wqk_sb[:, cc, :],
                                xt[:, cc, 256 * h2:256 * (h2 + 1)],
                                start=(cc == 0), stop=(cc == CS - 1))
                    for sc in range(TC):
                        for cc in range(CS):
                            nc.tensor.matmul(
                                pv4[:, sc // 2, sc % 2, :],
                                xt[:, cc, 128 * sc:128 * (sc + 1)],
                                wv_sb[:, cc, :],
                                start=(cc == 0), stop=(cc == CS - 1))

                    if r == 0:
                        qk2s[b // 2] = sb_qk.tile([128, 2, T], dt.bfloat16,
                                                  name=f"qk{b // 2}",
                                                  tag="qk")
                    qk2 = qk2s[b // 2]
                    nc.vector.tensor_copy(
                        qk2[:, r, :].rearrange("p (a c) -> p a c", a=2),
                        pqk2)
                    if r == 1:
                        kt2 = sb_qk.tile([64, 2, T], dt.bfloat16,
                                         name=f"kt2{b // 2}", tag="kt2")
                        kt2s[b // 2] = kt2
                        nc.sync.dma_start(kt2[:], qk2[64:128])

                    v1 = sb_v.tile([128, TC, H + 1], dt.bfloat16,
                                   name=f"v1{b}", tag="v1")
                    v1s[b] = v1
                    nc.vector.tensor_copy(
                        v1[:, :, 0:H].rearrange("p (a s) h -> p a s h", a=2),
                        pv4)
                    nc.gpsimd.memset(v1[:, :, H], 1.0)

                # ---- stage SIM (batch k-SIM_LAG): sim matmuls + exp + mask
                if 0 <= k - SIM_LAG < BL:
                    b = k - SIM_LAG
                    r = b % 2
                    qT = qk2s[b // 2][0:64, r, :]
                    kT = kt2s[b // 2][:, r, :]
                    psim = pt_all[:, r, 0:1280]
                    for si in range(TC):
                        d = 128 * si
                        nc.tensor.matmul(
                            psim[:, d:d + 128],
                            kT[:, 128 * si:128 * (si + 1)],
                            qT[:, 128 * si:128 * (si + 1)],
                            start=True, stop=True)
                    nc.tensor.matmul(psim[:, OD0:OD0 + 384],
                                     kT[:, 0:128], qT[:, 128:512],
                                     start=True, stop=True)
                    nc.tensor.matmul(psim[:, OD1:OD1 + 256],
                                     kT[:, 128:256], qT[:, 256:512],
                                     start=True, stop=True)
                    nc.tensor.matmul(psim[:, OD2:OD2 + 128],
                                     kT[:, 256:384], qT[:, 384:512],
                                     start=True, stop=True)
                    pt = sb_p.tile([128, 1280], dt.bfloat16, name=f"pt{b}",
                                   tag="pt")
                    pts[b] = pt
                    nc.scalar.activation(pt[:], psim[:],
                                         mybir.ActivationFunctionType.Exp,
                                         scale=float(INV_SQRT_H))
                    dv = pt[:, 0:512].rearrange("p (si t) -> p si t",
                                                si=TC)
                    nc.gpsimd.affine_select(
                        dv, dv, [[0, TC], [1, 128]],
                        mybir.AluOpType.is_ge, 0.0,
                        base=0, channel_multiplier=-1)

                # ---- stage AV (batch k-AV_LAG): AV matmuls incl. rowsum col
                if 0 <= k - AV_LAG < BL:
                    b = k - AV_LAG
                    pt = pts[b]
                    v1 = v1s[b]
                    pav = pavd[:, b % 2]
                    for ci in range(TC):
                        for si in range(ci + 1):
                            off = _blk(si, ci)
                            nc.tensor.matmul(
                                pav[:, ci, :],
                                pt[:, off:off + 128],
                                v1[:, si, :],
                                start=(si == 0), stop=(si == ci))

    nc.compile()
    return nc


_CACHED = None


def _get_program():
    global _CACHED
    if _CACHED is None:
        _CACHED = _build_program()
    return _CACHED


def _host_inputs(Wq, Wk, Wv):
    bf16 = ml_dtypes.bfloat16
    wq = np.asarray(Wq, np.float32)
    wk = np.asarray(Wk, np.float32)
    wv = np.asarray(Wv, np.float32)
    wqk = np.concatenate([wq, wk], axis=1)          # [C, 128]
    consts = {
        "wqk": np.ascontiguousarray(wqk.reshape(CS, 128, 128)).astype(bf16),
        "wv": np.ascontiguousarray(wv.reshape(CS, 128, H)).astype(bf16),
    }
    return consts


def _in_maps(input_embeddings, Wq, Wk, Wv):
    bf16 = ml_dtypes.bfloat16
    x = np.asarray(input_embeddings, np.float32)
    xt = np.ascontiguousarray(x.transpose(0, 2, 1)).astype(bf16)  # [B, C, T]
    xt = xt.reshape(B, CS, 128, T)
    consts = _host_inputs(Wq, Wk, Wv)
    in_maps = []
    for c in range(N_CORES):
        m = {"xt": xt[c * BL:(c + 1) * BL]}
        m.update(consts)
        in_maps.append(m)
    return in_maps


def kernel(input_embeddings, Wq, Wk, Wv):
    from concourse.bass_utils import run_bass_kernel_spmd

    nc = _get_program()
    in_maps = _in_maps(input_embeddings, Wq, Wk, Wv)
    res = run_bass_kernel_spmd(nc, in_maps, core_ids=list(range(N_CORES)))
    outs = []
    for c in range(N_CORES):
        o = np.asarray(res.results[c]["out"], np.float32)
        # [BL//2, 128, 2, TC, H+1] -> [BL, T, H+1]
        o = o.transpose(0, 2, 3, 1, 4).reshape(BL, T, H + 1)
        outs.append(o)
    out = np.concatenate(outs, axis=0)
    return out[:, :, 0:H] / out[:, :, H:H + 1]


if __name__ == "__main__":
    rng = np.random.default_rng(0)
    x = rng.standard_normal((B, T, C)).astype(np.float32)
    wq = (rng.standard_normal((C, H)) / 16).astype(np.float32)
    wk = (rng.standard_normal((C, H)) / 16).astype(np.float32)
    wv = (rng.standard_normal((C, H)) / 16).astype(np.float32)
    out = kernel(x, wq, wk, wv)
    print("out", out.shape, out.dtype)


# revision 8
# speedup vs baseline: 1.5915x; 1.5915x over previous
"""Single-head causal attention (B=128, T=512, C=256, H=64) on 8 trn2 cores.

Data-parallel: 16 batches per core. Host pre-transposes/casts x to bf16
xT [C, T] per batch (host prep is free), so the device does no transposes
and no cast-DMAs. Per batch b, software-pipelined across slots:

  slot b   : load xT (HWDGE, ramped 1/1/2/2.. batch chunks);
             pqk = [Wq|Wk]^T @ xT (one merged 128-wide PE pass into bank 5);
             pv = xT-chunks @ Wv (128-col halves of banks 6/7); qk copy ->
             qk2 pair tile (DVE); v1 copy (DVE) into [128,4,65] tile whose
             65th column is set to 1.0 (Pool memset) so the AV matmuls
             produce the softmax denominator for free.
  odd slots: one SBUF->SBUF DMA shifts the k-halves of BOTH batches of the
             qk2 pair to partitions 0-63 (PE needs stationary+moving at the
             same base partition).
  slot b+4 : 7 sim matmuls -> packed block-causal psim [128,1280] fp32;
             one ACT exp over all 1280 cols -> pT bf16; lower triangle of
             the 4 diag blocks zeroed in-place via gpsimd affine_select.
  slot b+6 : 10 AV matmuls (pT blocks stationary, v1 [128,65] moving) ->
             pav [128,4,65] incl. rowsum column (banks 6/7 by parity).
  slot b+7/8: DVE drains pav -> osb [128,2,4,65] bf16 (one parity per
             slot); fully-contiguous out DMA (1040B/partition descriptors)
             every even slot. Host unscrambles, divides by the rowsum
             column and upcasts to fp32.

PSUM dependency fencing is bank-granular, so the layout keeps each 2KB
bank single-cadence (one [128, 4096] tile, manually sliced):
  bank0 diagA | bank1 od0A+od2A | bank2 od1A+od1B | bank3 diagB
  bank4 od0B+od2B | bank5 pqk | bank6 pavA+pv01 | bank7 pavB+pv23
psim-A is flat cols 0:1280, psim-B flat cols 1280:2560 (od1B leads), so
each exp is one contiguous [128,1280] ACT op; bank2 alternates od1A/od1B
reads+writes on alternate slots (1-slot slack).  An ACT exp warmup at t=0
pulls the 1.3us activation-table load into the initial DMA dead time.
"""
import numpy as np
import ml_dtypes

B, T, C, H = 128, 512, 256, 64
N_CORES = 8
BL = B // N_CORES          # batches per core
TC = T // 128              # 4 t-chunks
CS = C // 128              # 2 c-subtiles
INV_SQRT_H = 1.0 / np.sqrt(H)

XCH = 2                    # batches per x-load DMA
SIM_LAG = 4                # sim of batch b runs in slot b+SIM_LAG
AV_LAG = 6                 # AV of batch b runs in slot b+AV_LAG
DRAIN_LAG = 7              # drain of batch b runs in slot b+DRAIN_LAG

# flat psum column offsets (fp32 cols; bank = 512 cols)
PQK_OFF = 2560             # bank5: pqk [128, 512]
PAV_A, PAV_B = 3072, 3584  # banks 6/7: pav [4, 65] per parity
PV_OFF = 3332              # banks 6/7 at +260: pv halves [2, 128] stride 512


def _blk(si, ci, par):
    """pT column offset of [128,128] block (s-chunk si, t-chunk ci).

    Parity A pt layout: diags@0 od0@512 od2@896 od1@1024.
    Parity B pt layout: od1@0 diags@256 od0@768 od2@1152.
    """
    base = 0 if par == 0 else 256
    if si == ci:
        return base + 128 * si
    if si == 0:
        return base + 512 + 128 * (ci - 1)
    if si == 1:  # od1
        return (1024 if par == 0 else 0) + 128 * (ci - 2)
    assert si == 2 and ci == 3
    return base + 896


def _build_program():
    import concourse.tile as tile
    from concourse import bacc, mybir

    dt = mybir.dt
    nc = bacc.Bacc("TRN2", target_bir_lowering=False, debug=False,
                   enable_asserts=False, num_devices=N_CORES)

    xt_d = nc.dram_tensor("xt", [BL, CS, 128, T], dt.bfloat16,
                          kind="ExternalInput").ap()
    wqk_d = nc.dram_tensor("wqk", [CS, 128, 128], dt.bfloat16,
                           kind="ExternalInput").ap()
    wv_d = nc.dram_tensor("wv", [CS, 128, H], dt.bfloat16,
                          kind="ExternalInput").ap()
    out_d = nc.dram_tensor("out", [BL // 2, 128, 2, TC, H + 1], dt.bfloat16,
                           kind="ExternalOutput").ap()

    with tile.TileContext(nc) as tc:
        from contextlib import ExitStack
        ctx = ExitStack()
        with ctx:
            consts = ctx.enter_context(tc.tile_pool(name="consts", bufs=1))
            ps = ctx.enter_context(tc.tile_pool(name="ps", bufs=1,
                                                space="PSUM"))
            # whole PSUM as one tile, sliced manually per the bank map above
            pt_all = ps.tile([128, 4096], dt.float32, name="psall")
            sb_x = ctx.enter_context(tc.tile_pool(name="sb_x", bufs=3))
            sb_qk = ctx.enter_context(tc.tile_pool(name="sb_qk", bufs=4))
            sb_p = ctx.enter_context(tc.tile_pool(name="sb_p", bufs=4))
            sb_v = ctx.enter_context(tc.tile_pool(name="sb_v", bufs=8))
            sb_o = ctx.enter_context(tc.tile_pool(name="sb_o", bufs=3))

            # ACT activation-table warmup so the implicit table load happens
            # during the initial DMA dead-time, not before the first real exp.
            warm = consts.tile([128, 1], dt.float32)
            nc.gpsimd.memset(warm[:], 0.0)
            warm2 = consts.tile([128, 1], dt.bfloat16)
            nc.scalar.activation(warm2[:], warm[:],
                                 mybir.ActivationFunctionType.Exp, scale=1.0)

            wqk_sb = consts.tile([128, CS, 128], dt.bfloat16)
            nc.sync.dma_start(wqk_sb[:], wqk_d.rearrange("cs p m -> p cs m"))
            wv_sb = consts.tile([128, CS, H], dt.bfloat16)
            nc.sync.dma_start(wv_sb[:], wv_d.rearrange("cs p h -> p cs h"))

            # manual psum slices
            pqk = pt_all[:, PQK_OFF:PQK_OFF + 512]
            # pv halves: banks 6/7 at +260, [128, 2, 2, 64] (bank stride 512)
            pv4 = pt_all.rearrange("p (a c) -> p a c", a=8)[
                :, 6:8, 260:388].rearrange("p a (s c) -> p a s c", s=2)
            pavA = pt_all[:, PAV_A:PAV_A + 260].rearrange(
                "p (tc h) -> p tc h", tc=TC)
            pavB = pt_all[:, PAV_B:PAV_B + 260].rearrange(
                "p (tc h) -> p tc h", tc=TC)

            xts = [None] * BL
            qk2s = [None] * (BL // 2)
            kt2s = [None] * (BL // 2)
            pts = [None] * BL
            v1s = [None] * BL
            osbs = [None] * (BL // 2)

            # x-load chunks: [1,1] first (fast pipeline fill), then pairs
            chunks = [(0, 1), (1, 2)] + [(c, c + 2) for c in range(2, BL, 2)]
            chunk_issue_slot = [max(0, lo - 2) for lo, hi in chunks]

            n_slots = BL + DRAIN_LAG + 1
            next_chunk = 0
            for k in range(n_slots):
                # ---- x prefetch
                while (next_chunk < len(chunks)
                       and chunk_issue_slot[next_chunk] <= k):
                    lo, hi = chunks[next_chunk]
                    xt2 = sb_x.tile([128, XCH, CS, T], dt.bfloat16,
                                    name=f"xt2_{next_chunk}", tag="xt2")
                    nc.sync.dma_start(
                        xt2[:, 0:hi - lo],
                        xt_d[lo:hi].rearrange("b cs p t -> p b cs t"))
                    for j in range(hi - lo):
                        xts[lo + j] = xt2[:, j]
                    next_chunk += 1

                # ---- stage P1 (batch k): qk + v matmuls, copies, ones
                if k < BL:
                    b = k
                    r = b % 2
                    xt = xts[b]
                    for cc in range(CS):
                        nc.tensor.matmul(pqk[:], wqk_sb[:, cc, :],
                                         xt[:, cc, :],
                                         start=(cc == 0), stop=(cc == CS - 1))
                    for sc in range(TC):
                        for cc in range(CS):
                            nc.tensor.matmul(
                                pv4[:, sc // 2, sc % 2, :],
                                xt[:, cc, 128 * sc:128 * (sc + 1)],
                                wv_sb[:, cc, :],
                                start=(cc == 0), stop=(cc == CS - 1))

                    if r == 0:
                        qk2s[b // 2] = sb_qk.tile([128, 2, T], dt.bfloat16,
                                                  name=f"qk{b // 2}",
                                                  tag="qk")
                    qk2 = qk2s[b // 2]
                    nc.vector.tensor_copy(qk2[:, r, :], pqk[:])
                    if r == 1:
                        kt2 = sb_qk.tile([64, 2, T], dt.bfloat16,
                                         name=f"kt2{b // 2}", tag="kt2")
                        kt2s[b // 2] = kt2
                        nc.sync.dma_start(kt2[:], qk2[64:128])

                    v1 = sb_v.tile([128, TC, H + 1], dt.bfloat16,
                                   name=f"v1{b}", tag="v1")
                    v1s[b] = v1
                    nc.vector.tensor_copy(
                        v1[:, :, 0:H].rearrange("p (a s) h -> p a s h", a=2),
                        pv4)
                    nc.gpsimd.memset(v1[:, :, H], 1.0)

                # ---- drain (batch k-DRAIN_LAG): pav -> osb; out DMA per pair
                if 0 <= k - DRAIN_LAG < BL:
                    b = k - DRAIN_LAG
                    r = b % 2
                    if r == 0:
                        osbs[b // 2] = sb_o.tile([128, 2, TC, H + 1],
                                                 dt.bfloat16,
                                                 name=f"osb{b // 2}",
                                                 tag="osb")
                    osb = osbs[b // 2]
                    nc.vector.tensor_copy(osb[:, r], pavA if r == 0 else pavB)
                    if r == 1:
                        nc.sync.dma_start(out_d[b // 2], osb[:])

                # ---- stage SIM (batch k-SIM_LAG): sim matmuls + exp + mask
                if 0 <= k - SIM_LAG < BL:
                    b = k - SIM_LAG
                    r = b % 2
                    qT = qk2s[b // 2][0:64, r, :]
                    kT = kt2s[b // 2][:, r, :]
                    psim = pt_all[:, 1280 * r:1280 * (r + 1)]
                    for si in range(TC):
                        d = _blk(si, si, r)
                        nc.tensor.matmul(
                            psim[:, d:d + 128],
                            kT[:, 128 * si:128 * (si + 1)],
                            qT[:, 128 * si:128 * (si + 1)],
                            start=True, stop=True)
                    d = _blk(0, 1, r)
                    nc.tensor.matmul(psim[:, d:d + 384],
                                     kT[:, 0:128], qT[:, 128:512],
                                     start=True, stop=True)
                    d = _blk(1, 2, r)
                    nc.tensor.matmul(psim[:, d:d + 256],
                                     kT[:, 128:256], qT[:, 256:512],
                                     start=True, stop=True)
                    d = _blk(2, 3, r)
                    nc.tensor.matmul(psim[:, d:d + 128],
                                     kT[:, 256:384], qT[:, 384:512],
                                     start=True, stop=True)
                    pt = sb_p.tile([128, 1280], dt.bfloat16, name=f"pt{b}",
                                   tag="pt")
                    pts[b] = pt
                    nc.scalar.activation(pt[:], psim[:],
                                         mybir.ActivationFunctionType.Exp,
                                         scale=float(INV_SQRT_H))
                    dbase = _blk(0, 0, r)
                    dv = pt[:, dbase:dbase + 512].rearrange(
                        "p (si t) -> p si t", si=TC)
                    nc.gpsimd.affine_select(
                        dv, dv, [[0, TC], [1, 128]],
                        mybir.AluOpType.is_ge, 0.0,
                        base=0, channel_multiplier=-1)

                # ---- stage AV (batch k-AV_LAG): AV matmuls incl. rowsum col
                if 0 <= k - AV_LAG < BL:
                    b = k - AV_LAG
                    r = b % 2
                    pt = pts[b]
                    v1 = v1s[b]
                    pav = pavA if r == 0 else pavB
                    for ci in range(TC):
                        for si in range(ci + 1):
                            off = _blk(si, ci, r)
                            nc.tensor.matmul(
                                pav[:, ci, :],
                                pt[:, off:off + 128],
                                v1[:, si, :],
                                start=(si == 0), stop=(si == ci))

    nc.compile()
    return nc


_CACHED = None


def _get_program():
    global _CACHED
    if _CACHED is None:
        _CACHED = _build_program()
    return _CACHED


def _host_inputs(Wq, Wk, Wv):
    bf16 = ml_dtypes.bfloat16
    wq = np.asarray(Wq, np.float32)
    wk = np.asarray(Wk, np.float32)
    wv = np.asarray(Wv, np.float32)
    wqk = np.concatenate([wq, wk], axis=1)          # [C, 128]
    consts = {
        "wqk": np.ascontiguousarray(wqk.reshape(CS, 128, 128)).astype(bf16),
        "wv": np.ascontiguousarray(wv.reshape(CS, 128, H)).astype(bf16),
    }
    return consts


def _in_maps(input_embeddings, Wq, Wk, Wv):
    bf16 = ml_dtypes.bfloat16
    x = np.asarray(input_embeddings, np.float32)
    xt = np.ascontiguousarray(x.transpose(0, 2, 1)).astype(bf16)  # [B, C, T]
    xt = xt.reshape(B, CS, 128, T)
    consts = _host_inputs(Wq, Wk, Wv)
    in_maps = []
    for c in range(N_CORES):
        m = {"xt": xt[c * BL:(c + 1) * BL]}
        m.update(consts)
        in_maps.append(m)
    return in_maps


def kernel(input_embeddings, Wq, Wk, Wv):
    from concourse.bass_utils import run_bass_kernel_spmd

    nc = _get_program()
    in_maps = _in_maps(input_embeddings, Wq, Wk, Wv)
    res = run_bass_kernel_spmd(nc, in_maps, core_ids=list(range(N_CORES)))
    outs = []
    for c in range(N_CORES):
        o = np.asarray(res.results[c]["out"], np.float32)
        # [BL//2, 128, 2, TC, H+1] -> [BL, T, H+1]
        o = o.transpose(0, 2, 3, 1, 4).reshape(BL, T, H + 1)
        outs.append(o)
    out = np.concatenate(outs, axis=0)
    return out[:, :, 0:H] / out[:, :, H:H + 1]


if __name__ == "__main__":
    rng = np.random.default_rng(0)
    x = rng.standard_normal((B, T, C)).astype(np.float32)
    wq = (rng.standard_normal((C, H)) / 16).astype(np.float32)
    wk = (rng.standard_normal((C, H)) / 16).astype(np.float32)
    wv = (rng.standard_normal((C, H)) / 16).astype(np.float32)
    out = kernel(x, wq, wk, wv)
    print("out", out.shape, out.dtype)
